# revision 97
# baseline (speedup 1.0000x reference)
"""MoE (E=4 experts, top-2 routing) forward pass on 8 Trainium2 NeuronCores.

Strategy: data-parallel over tokens (core i processes batch row i = 2048
tokens), with TRUE top-2 routing on-device: each token's expert pair comes
from the fp32 gate scores; tokens are compacted per expert with tile-major
slot ids built from three small PE matmuls (strict-upper-triangular
partition prefix + column-sum + broadcast) and scattered via one masked
multi-row indirect DMA per expert into per-expert DRAM batches; each
expert runs its 2-layer FFN over its ~C=1152-token batch (vs 2048 dense ->
~1.8x less matmul work); per-expert outputs are gathered back by slot and
accumulated with the softmax weights. Tile-major slot order makes slots
monotonic in token-tile index, so each expert's combine overlaps its OWN
compute using a host-validated pacing table (gather for token tile tt
issues once the first pace[e][tt] routed tiles are done).

Per routed tile (36 = 4 experts x 9 capacity tiles):
  xrtT = xbar DMA-transpose load of 128 gathered tokens (bf16, no PE)
  z    = x @ W1   PE bf16, fp32 PSUM (stationary = xrtT, moving = W1)
  u    = relu(LN1(z))   DVE bn_stats + ACT fused scale/bias+relu
  uT   PE transpose + ACT copy
  z2   = u @ W2   PE bf16
  y    = LN2(z2) -> bf16 -> DMA to yrt[C*e + slot]

Identity affine params (g=1, b=0 -- how this problem's inputs are built)
are folded away at kernel() call time by inspecting the numpy values; a
general fallback path applies them when nonzero. Routing/gating stays fp32
throughout (min top2/top3 score gap on this data ~2e-5). Capacity C and
the pacing table are input statistics computed on host; the device does
all routing. A different input recompiles with its own (C, pace).
"""

import threading
from contextlib import ExitStack

import numpy as np

import concourse.bass as bass
import concourse.mybir as mybir
import concourse.tile as tile
from concourse import bacc
from concourse.bass import ds, ts
from concourse.masks import make_identity, make_upper_triangular

F32 = mybir.dt.float32
BF16 = mybir.dt.bfloat16
I32 = mybir.dt.int32
AF = mybir.ActivationFunctionType
ALU = mybir.AluOpType
AX = mybir.AxisListType

P = 128
D = 1024
E = 4
KC = D // P  # contraction chunks per matmul
NCH = D // 512  # psum column chunks
LN_EPS = 1e-5
N_CORES = 8
OOB = 1 << 20  # slot offset masking unselected (expert, token) pairs


def _row1(ap):
    """Lift an AP to have a leading length-1 (partition) dim."""
    return bass.AP(tensor=ap.tensor, offset=ap.offset, ap=[[0, 1]] + list(ap.ap))


def _bcast_rows(ap_row, p=P):
    """Broadcast a [1, N]-ish DRAM AP across p partitions (step-0 partition dim)."""
    inner = [list(d) for d in ap_row.ap if d[1] != 1]
    return bass.AP(tensor=ap_row.tensor, offset=ap_row.offset, ap=[[0, p]] + inner)


def build_moe_nc(T=2048, C=1152, pace=None, scdep=None, rcap=None, flags=(),
                 num_devices=N_CORES):
    """pace[e][k]: number of expert-e routed tiles that must be complete
    before token-tile pair (2k, 2k+1) may gather expert e's output
    (host-validated upper bound). flags: the non-identity affine params."""
    TT = T // P
    R = C // P  # routed tiles per expert
    NIT = E * R  # total routed tiles
    has = set(flags)
    if pace is None:
        pace = tuple(tuple(R for _ in range(TT // 2)) for _ in range(E))
    if scdep is None:
        # conservative default: every load waits the expert's last scatter
        scdep = tuple(
            tuple(-1 if tt < TT - 1 else 0 for tt in range(TT)) for _ in range(E)
        )
        rcap = tuple((0, 0) for _ in range(E))
    nc = bacc.Bacc(
        "TRN2", target_bir_lowering=False, debug=False, num_devices=num_devices
    )

    x_d = nc.dram_tensor("x", [T, D], F32, kind="ExternalInput")
    gw_d = nc.dram_tensor("gate_W", [D, E], F32, kind="ExternalInput")
    gb_d = nc.dram_tensor("gate_b", [E], F32, kind="ExternalInput")
    w1_d = nc.dram_tensor("W1", [E, D, D], F32, kind="ExternalInput")
    b1_d = nc.dram_tensor("b1", [E, D], F32, kind="ExternalInput")
    g1_d = nc.dram_tensor("g1", [E, D], F32, kind="ExternalInput")
    be1_d = nc.dram_tensor("be1", [E, D], F32, kind="ExternalInput")
    w2_d = nc.dram_tensor("W2", [E, D, D], F32, kind="ExternalInput")
    b2_d = nc.dram_tensor("b2", [E, D], F32, kind="ExternalInput")
    g2_d = nc.dram_tensor("g2", [E, D], F32, kind="ExternalInput")
    be2_d = nc.dram_tensor("be2", [E, D], F32, kind="ExternalInput")
    out_d = nc.dram_tensor("out", [T, D], F32, kind="ExternalOutput")

    with tile.TileContext(nc) as tc:
        with ExitStack() as stack:
            ep = stack.enter_context
            const = ep(tc.tile_pool(name="const", bufs=1))
            drampx = ep(tc.tile_pool(name="dramx", bufs=1, space="DRAM"))
            drampy = ep(tc.tile_pool(name="dramy", bufs=1, space="DRAM"))
            xfp = ep(tc.tile_pool(name="xfp", bufs=3))
            xbfp = ep(tc.tile_pool(name="xbfp", bufs=1))
            xtgp = ep(tc.tile_pool(name="xtgp", bufs=2))
            routep = ep(tc.tile_pool(name="routep", bufs=1))
            gstp = ep(tc.tile_pool(name="gstp", bufs=1))
            top2p = ep(tc.tile_pool(name="top2p", bufs=2))
            wp = ep(tc.tile_pool(name="wp", bufs=1 if flags else 2))
            repp = ep(tc.tile_pool(name="repp", bufs=1 if flags else 2))
            bvep = ep(tc.tile_pool(name="bvep", bufs=2))
            xrtTp = ep(tc.tile_pool(name="xrtTp", bufs=3))
            workp = ep(tc.tile_pool(name="workp", bufs=3))
            statp = ep(tc.tile_pool(name="statp", bufs=3))
            accp = ep(tc.tile_pool(name="accp", bufs=TT))
            gp = ep(tc.tile_pool(name="gp", bufs=4))
            combp = ep(tc.tile_pool(name="combp", bufs=2))

            # ---- constants ----
            id_f32 = const.tile([P, P], F32)
            make_identity(nc, id_f32)
            id_bf16 = const.tile([P, P], BF16)
            make_identity(nc, id_bf16)
            utri = const.tile([P, P], F32)
            make_upper_triangular(nc, utri, val=1.0, diag=False)  # U[q,p]=1 iff q<p
            ones_bf = const.tile([1, P], BF16)
            nc.vector.memset(ones_bf, 1.0)
            ones_f32 = const.tile([1, P], F32)
            nc.vector.memset(ones_f32, 1.0)
            ones_col = const.tile([P, 1], F32)
            nc.vector.memset(ones_col, 1.0)
            eps_sb = const.tile([P, 1], F32)
            nc.vector.memset(eps_sb, LN_EPS)
            ce_pe = const.tile([P, E], F32)  # [0, C, 2C, 3C] per partition
            for e in range(E):
                nc.vector.memset(ce_pe[:, e : e + 1], float(C * e))

            gw_sb = const.tile([P, KC, E], F32)
            nc.sync.dma_start(out=gw_sb, in_=gw_d.rearrange("(c p) e -> p c e", p=P))
            gb_sb = const.tile([1, E], F32)
            nc.sync.dma_start(out=gb_sb, in_=_row1(gb_d[:]))

            # routed token batches (bf16): per-expert tensors so expert e's
            # loads wait only on expert e's scatter
            xrt = [
                drampx.tile([C, D], BF16, tag=f"xrt{e}", name=f"xrt{e}")
                for e in range(E)
            ]
            yrt = drampy.tile([E * C, D], BF16, tag="yrt", name="yrt")

            # ---- expert weight loads (bf16 casting DMA on gpsimd) ----
            w1sb = {}
            w2sb = {}

            def load_w(e, which):
                # 4 chunks so no single transfer holds the DMA engines long
                # enough to starve the latency-critical xrtT loads
                src = w1_d if which == 1 else w2_d
                t = wp.tile([P, KC, D], BF16, tag=f"w{which}", name=f"w{which}_{e}")
                h = KC // 4
                for q in range(4):
                    nc.gpsimd.dma_start(
                        out=t[:, q * h : (q + 1) * h, :],
                        in_=src[e, ds(q * h * P, h * P), :].rearrange(
                            "(c p) n -> p c n", p=P
                        ),
                    )
                (w1sb if which == 1 else w2sb)[e] = t

            bves = {}
            reps = {}

            def load_bve(e):
                if not (has & {"b1", "b2"}):
                    return
                bve = bvep.tile([1, 2, D], BF16, tag="bve", name=f"bve_{e}")
                nc.gpsimd.dma_start(out=bve[:, 0, :], in_=_row1(b1_d[e, :]))
                nc.gpsimd.dma_start(out=bve[:, 1, :], in_=_row1(b2_d[e, :]))
                bves[e] = bve

            def load_reps(e):
                if not (has & {"g1", "be1", "g2", "be2"}):
                    return
                tiles = {}
                for nm, src in (("g1", g1_d), ("be1", be1_d), ("g2", g2_d), ("be2", be2_d)):
                    if nm in has:
                        t = repp.tile([P, D], BF16, tag=nm, name=f"{nm}_{e}")
                        nc.gpsimd.dma_start(out=t, in_=_bcast_rows(src[e : e + 1, :]))
                        tiles[nm] = t
                reps[e] = tiles

            # ---- prologue: stream x, fp32 gating (PE software-pipelined) ----
            pre_ctx = tc.tile_pool(name="prep", bufs=2, space="PSUM")
            prep = pre_ctx.__enter__()
            po_ctx = tc.tile_pool(name="pop", bufs=1, space="PSUM")
            pop = po_ctx.__enter__()

            xbf = xbfp.tile([P, TT, D], BF16, tag="xbf", name="xbf")
            scores_sb = const.tile([P, TT, E], F32)
            indT = routep.tile([P, E, TT], F32, tag="indT")
            nonselT = routep.tile([P, E, TT], F32, tag="nonselT")
            w_sb = gstp.tile([P, TT, E], F32, tag="w_sb")
            giSC = routep.tile([P, E * TT], I32, tag="giSC")
            giSCv = giSC.rearrange("p (e t) -> p e t", e=E)
            giE = routep.tile([P, E * TT], I32, tag="giE")
            giEv = giE.rearrange("p (e t) -> p e t", e=E)
            runt = routep.tile([1, E, 1], F32, tag="runt")
            nc.vector.memset(runt, 0.0)
            deferred_scat = []
            xtgs = {}

            def scatter(e, tt):
                # offsets stay [P, 1]: multi-offset indirect DMAs give wrong
                # results on real HW. Out window [0:P] bounds the per-tile
                # rows so the cost model matches the real transfer.
                # dep_tracking_offset maps this scatter to the routed-tile
                # range it COMPLETES (host-validated, min over cores), so
                # loads wait on exactly the right scatter; the single
                # in-order SWDGE queue guarantees all earlier scatters are
                # done by then. Unmapped scatters get a unique range beyond
                # any reader so no false WAW chains form.
                base = xrt[e][0:P, :]
                sd = scdep[e][tt]
                fake = (sd * P * D) if sd >= 0 else (C * D + tt * P * D)
                outap = bass.AP(
                    tensor=base.tensor, offset=base.offset, ap=base.ap,
                    dep_tracking_offset=fake,
                )
                nc.gpsimd.indirect_dma_start(
                    out=outap,
                    out_offset=bass.IndirectOffsetOnAxis(
                        ap=giSCv[:, e, tt : tt + 1], axis=0
                    ),
                    in_=xbf[:, tt, :],
                    in_offset=None,
                    bounds_check=C - 1,
                    oob_is_err=False,
                )

            def route_chunk(a):
                # tile-major slots only depend on token tiles <= tt, so the
                # slot math and expert-0 scatters run incrementally per
                # 4-tile chunk, overlapped with the rest of the gating stream
                n = 4
                iv = indT[:, :, a : a + n]  # [P, E, n]
                cs_ps = pop.tile([1, E, n], F32, tag="cs")
                nc.tensor.matmul(cs_ps, ones_col, iv, start=True, stop=True)
                csb = top2p.tile([1, E, n], F32, tag="csb")
                nc.vector.tensor_copy(out=csb, in_=cs_ps)
                i1 = top2p.tile([1, E, n], F32, tag="i1")
                nc.vector.tensor_tensor(
                    out=i1[:, :, 1:], in0=csb[:, :, 1:], in1=csb[:, :, : n - 1],
                    op=ALU.add,
                )
                nc.vector.tensor_copy(out=i1[:, :, :1], in_=csb[:, :, :1])
                incl = top2p.tile([1, E, n], F32, tag="incl")
                nc.vector.tensor_tensor(
                    out=incl[:, :, 2:], in0=i1[:, :, 2:], in1=i1[:, :, : n - 2],
                    op=ALU.add,
                )
                nc.vector.tensor_copy(out=incl[:, :, :2], in_=i1[:, :, :2])
                tb = top2p.tile([1, E, n], F32, tag="tb")
                nc.vector.tensor_tensor(out=tb, in0=incl, in1=csb, op=ALU.subtract)
                nc.vector.tensor_tensor(
                    out=tb, in0=tb, in1=runt.broadcast_to((1, E, n)), op=ALU.add
                )
                nc.vector.tensor_tensor(
                    out=runt, in0=runt, in1=incl[:, :, n - 1 : n], op=ALU.add
                )
                slotp = pop.tile([P, E, n], F32, tag="slotp")
                nc.tensor.matmul(slotp, utri, iv, start=True, stop=False)
                nc.tensor.matmul(slotp, ones_f32, tb, start=False, stop=True)
                sloc = top2p.tile([P, E, n], F32, tag="sloc")
                nc.vector.tensor_copy(out=sloc, in_=slotp)
                scf = top2p.tile([P, E, n], F32, tag="scf")
                nc.vector.scalar_tensor_tensor(
                    out=scf, in0=nonselT[:, :, a : a + n], scalar=float(OOB),
                    in1=sloc, op0=ALU.mult, op1=ALU.add,
                )
                nc.vector.tensor_copy(out=giSCv[:, :, a : a + n], in_=scf)
                gf = top2p.tile([P, E, n], F32, tag="gf")
                nc.vector.tensor_tensor(
                    out=gf, in0=scf, in1=ce_pe.broadcast_to((P, E, n)), op=ALU.add
                )
                nc.vector.tensor_copy(out=giEv[:, :, a : a + n], in_=gf)
                # expert 0's scatters go out immediately (its compute starts
                # first); experts 1-3 are deferred so expert 0's chain owns
                # the Pool desc-gen throughput
                for tt in range(a, a + n):
                    scatter(0, tt)
                for e in range(1, E):
                    for tt in range(a, a + n):
                        deferred_scat.append((e, tt))

            def top2_chunk(a, b):
                # top-2 softmax over E for token tiles [a, b): emitted as the
                # scores arrive so the whole thing hides under the x stream
                n = b - a
                sl = scores_sb[:, a:b, :]
                m1c = top2p.tile([P, n], F32, tag="m1")
                nc.vector.tensor_reduce(out=m1c, in_=sl, axis=AX.X, op=ALU.max)
                m1bc = m1c.broadcast_to((P, n, E))
                ind1c = top2p.tile([P, n, E], F32, tag="ind1")
                nc.vector.tensor_tensor(out=ind1c, in0=sl, in1=m1bc, op=ALU.is_equal)
                smtc = top2p.tile([P, n, E], F32, tag="smt")
                nc.vector.scalar_tensor_tensor(
                    out=smtc, in0=ind1c, scalar=-1e30, in1=sl,
                    op0=ALU.mult, op1=ALU.add,
                )
                m2c = top2p.tile([P, n], F32, tag="m2")
                nc.vector.tensor_reduce(out=m2c, in_=smtc, axis=AX.X, op=ALU.max)
                m2bc = m2c.broadcast_to((P, n, E))
                indv = indT[:, :, a:b].rearrange("p e t -> p t e")
                nc.vector.tensor_tensor(out=indv, in0=sl, in1=m2bc, op=ALU.is_ge)
                nc.vector.tensor_tensor(
                    out=nonselT[:, :, a:b].rearrange("p e t -> p t e"),
                    in0=sl, in1=m2bc, op=ALU.is_lt,
                )
                ddc = top2p.tile([P, n, E], F32, tag="dd")
                nc.vector.tensor_tensor(out=ddc, in0=sl, in1=m1bc, op=ALU.subtract)
                exc = top2p.tile([P, n, E], F32, tag="ex")
                nc.scalar.activation(out=exc, in_=ddc, func=AF.Exp)
                enc = top2p.tile([P, n, E], F32, tag="en")
                nc.vector.tensor_tensor(out=enc, in0=exc, in1=indv, op=ALU.mult)
                zsc = top2p.tile([P, n], F32, tag="zs")
                nc.vector.tensor_reduce(out=zsc, in_=enc, axis=AX.X, op=ALU.add)
                rzc = top2p.tile([P, n], F32, tag="rz")
                nc.vector.reciprocal(out=rzc, in_=zsc)
                nc.vector.tensor_tensor(
                    out=w_sb[:, a:b, :], in0=enc,
                    in1=rzc.broadcast_to((P, n, E)), op=ALU.mult,
                )

            def gate_mm(tt):
                gps = prep.tile([P, E], F32, tag="gate")
                for c in range(KC):
                    nc.tensor.matmul(
                        gps,
                        xtgs[tt][:, ts(c, P)],
                        gw_sb[:, c, :],
                        start=(c == 0),
                        stop=(c == KC - 1 and "gb" not in has),
                    )
                if "gb" in has:
                    nc.tensor.matmul(gps, ones_f32, gb_sb, start=False, stop=True)
                nc.vector.tensor_copy(out=scores_sb[:, tt, :], in_=gps)
                del xtgs[tt]

            for tt in range(TT):
                xf = xfp.tile([P, D], F32, tag="xf")
                nc.sync.dma_start(out=xf, in_=x_d[ts(tt, P), :])
                nc.scalar.copy(out=xbf[:, tt, :], in_=xf)
                tp = prep.tile([P, D], F32, tag="tp")
                for c in range(KC):
                    nc.tensor.transpose(tp[:, ts(c, P)], xf[:, ts(c, P)], id_f32)
                xtg = xtgp.tile([P, D], F32, tag="xtg")
                nc.vector.tensor_copy(out=xtg, in_=tp)
                xtgs[tt] = xtg
                # gate matmuls run one tile behind the transposes so the PE
                # never waits on the DVE psum->sbuf copy
                if tt >= 1:
                    gate_mm(tt - 1)
                    if tt % 4 == 0:
                        top2_chunk(tt - 4, tt)
                        route_chunk(tt - 4)
                if tt == TT - 1:
                    gate_mm(tt)
                    top2_chunk(TT - 4, TT)
                    route_chunk(TT - 4)
                    # W transfers staged off the gating-critical x stream
                    with tc.tile_wait_until(0.024):
                        load_w(0, 1)
                    with tc.tile_wait_until(0.030):
                        load_w(0, 2)
                        load_bve(0)
                        load_reps(0)
                    with tc.tile_wait_until(0.060):
                        load_w(1, 1)
                        load_w(1, 2)
                        load_bve(1)
                        load_reps(1)

            # experts 1-3's scatters, deferred past expert 0's p-state ramp
            # window AND spread per expert so no single compute window
            # carries more than one expert's scatter load (each expert's
            # batch is only needed when its compute starts)
            with tc.tile_wait_until(0.064):
                for e, tt in deferred_scat:
                    if e == 1:
                        scatter(e, tt)
            with tc.tile_wait_until(0.130):
                for e, tt in deferred_scat:
                    if e == 2:
                        scatter(e, tt)
            with tc.tile_wait_until(0.200):
                for e, tt in deferred_scat:
                    if e == 3:
                        scatter(e, tt)

            po_ctx.__exit__(None, None, None)
            pre_ctx.__exit__(None, None, None)

            # ---- routed expert compute, software-pipelined by one tile ----
            zp_ctx = tc.tile_pool(name="zp", bufs=2, space="PSUM")
            zp = zp_ctx.__enter__()
            z2p_ctx = tc.tile_pool(name="z2p", bufs=1, space="PSUM")
            z2p = z2p_ctx.__enter__()
            utp_ctx = tc.tile_pool(name="utp", bufs=2, space="PSUM")
            utp = utp_ctx.__enter__()

            xrtT = {}
            zt = {}
            ut = {}
            uts = {}
            z2t = {}

            def s_load(j):
                e, r = divmod(j, R)
                t = xrtTp.tile([P, KC, P], BF16, tag="xrtT", name=f"xrtT_{j}")
                src_ap = xrt[e][ds(r * P, P), :]
                if r > rcap[e][0]:
                    # rows past the min-core coverage: depend on the LAST
                    # scatter of this expert (in-order queue covers every
                    # core's writes by then)
                    src_ap = bass.AP(
                        tensor=src_ap.tensor, offset=src_ap.offset,
                        ap=src_ap.ap,
                        dep_tracking_offset=rcap[e][1] * P * D,
                    )
                nc.sync.dma_start_transpose(out=t, in_=src_ap)
                xrtT[j] = t

            def s_z(j):
                e, r = divmod(j, R)
                z = zp.tile([P, D], F32, tag="z", name=f"z_{j}")
                last = KC - 1
                for c in range(KC):
                    for n in range(NCH):
                        nc.tensor.matmul(
                            z[:, ds(n * 512, 512)],
                            xrtT[j][:, c, :],
                            w1sb[e][:, c, ds(n * 512, 512)],
                            start=(c == 0),
                            stop=(c == last and "b1" not in has),
                        )
                if "b1" in has:
                    for n in range(NCH):
                        nc.tensor.matmul(
                            z[:, ds(n * 512, 512)],
                            ones_bf,
                            bves[e][:, 0, ds(n * 512, 512)],
                            start=False,
                            stop=True,
                        )
                zt[j] = z
                del xrtT[j]

            def s_ln1(j):
                e, r = divmod(j, R)
                z = zt[j]
                st1 = statp.tile([P, 2, 6], F32, tag="st1")
                nc.vector.bn_stats(out=st1[:, 0, :], in_=z[:, 0:512])
                nc.vector.bn_stats(out=st1[:, 1, :], in_=z[:, 512:1024])
                mv1 = statp.tile([P, 2], F32, tag="mv1")
                nc.vector.bn_aggr(out=mv1, in_=st1)
                sd1 = statp.tile([P, 1], F32, tag="sd1")
                nc.scalar.activation(
                    out=sd1, in_=mv1[:, 1:2], func=AF.Sqrt, bias=eps_sb
                )
                rs1 = statp.tile([P, 1], F32, tag="rs1")
                nc.vector.reciprocal(out=rs1, in_=sd1)
                nmr1 = statp.tile([P, 1], F32, tag="nmr1")
                nc.vector.tensor_scalar(
                    out=nmr1,
                    in0=mv1[:, 0:1],
                    scalar1=rs1,
                    scalar2=-1.0,
                    op0=ALU.mult,
                    op1=ALU.mult,
                )
                simple = not (has & {"g1", "be1"})
                u = workp.tile([P, D], BF16, tag="u", name=f"u_{j}")
                if simple:
                    nc.scalar.activation(
                        out=u, in_=z, func=AF.Relu, bias=nmr1, scale=rs1
                    )
                else:
                    n1 = workp.tile([P, D], F32, tag="ng")
                    nc.scalar.activation(
                        out=n1, in_=z, func=AF.Identity, bias=nmr1, scale=rs1
                    )
                    if "g1" in has:
                        nc.vector.tensor_tensor(
                            out=n1, in0=n1, in1=reps[e]["g1"], op=ALU.mult
                        )
                    if "be1" in has:
                        nc.gpsimd.tensor_tensor(
                            out=n1, in0=n1, in1=reps[e]["be1"], op=ALU.add
                        )
                    nc.scalar.activation(out=u, in_=n1, func=AF.Relu)
                ut[j] = u
                del zt[j]

            def s_tu(j):
                u = ut[j]
                utps = utp.tile([P, D], BF16, tag="uT", name=f"uT_{j}")
                for c in range(KC):
                    nc.tensor.transpose(utps[:, ts(c, P)], u[:, ts(c, P)], id_bf16)
                t = workp.tile([P, KC, P], BF16, tag="uTs", name=f"uTs_{j}")
                nc.scalar.copy(out=t, in_=utps.rearrange("p (c q) -> p c q", c=KC))
                uts[j] = t
                del ut[j]

            def s_z2(j):
                e, r = divmod(j, R)
                z2 = z2p.tile([P, D], F32, tag="z2", name=f"z2_{j}")
                last = KC - 1
                for c in range(KC):
                    for n in range(NCH):
                        nc.tensor.matmul(
                            z2[:, ds(n * 512, 512)],
                            uts[j][:, c, :],
                            w2sb[e][:, c, ds(n * 512, 512)],
                            start=(c == 0),
                            stop=(c == last and "b2" not in has),
                        )
                if "b2" in has:
                    for n in range(NCH):
                        nc.tensor.matmul(
                            z2[:, ds(n * 512, 512)],
                            ones_bf,
                            bves[e][:, 1, ds(n * 512, 512)],
                            start=False,
                            stop=True,
                        )
                z2t[j] = z2
                del uts[j]

            def s_ln2(j):
                e, r = divmod(j, R)
                z2 = z2t[j]
                st2 = statp.tile([P, 2, 6], F32, tag="st2")
                nc.vector.bn_stats(out=st2[:, 0, :], in_=z2[:, 0:512])
                nc.vector.bn_stats(out=st2[:, 1, :], in_=z2[:, 512:1024])
                mv2 = statp.tile([P, 2], F32, tag="mv2")
                nc.vector.bn_aggr(out=mv2, in_=st2)
                sd2 = statp.tile([P, 1], F32, tag="sd2")
                nc.scalar.activation(
                    out=sd2, in_=mv2[:, 1:2], func=AF.Sqrt, bias=eps_sb
                )
                rs2 = statp.tile([P, 1], F32, tag="rs2")
                nc.vector.reciprocal(out=rs2, in_=sd2)
                nmr2 = statp.tile([P, 1], F32, tag="nmr2")
                nc.vector.tensor_scalar(
                    out=nmr2,
                    in0=mv2[:, 0:1],
                    scalar1=rs2,
                    scalar2=-1.0,
                    op0=ALU.mult,
                    op1=ALU.mult,
                )
                simple = not (has & {"g2", "be2"})
                y = workp.tile([P, D], BF16, tag="y", name=f"y_{j}")
                if simple:
                    nc.scalar.activation(
                        out=y, in_=z2, func=AF.Identity, bias=nmr2, scale=rs2
                    )
                else:
                    n2 = workp.tile([P, D], F32, tag="ng")
                    nc.scalar.activation(
                        out=n2, in_=z2, func=AF.Identity, bias=nmr2, scale=rs2
                    )
                    if "g2" in has:
                        nc.vector.tensor_tensor(
                            out=n2, in0=n2, in1=reps[e]["g2"], op=ALU.mult
                        )
                    if "be2" in has:
                        nc.gpsimd.tensor_tensor(
                            out=n2, in0=n2, in1=reps[e]["be2"], op=ALU.add
                        )
                    nc.scalar.copy(out=y, in_=n2)
                del z2t[j]
                # on the ACT hwdge queue: ACT produced y just above, and this
                # keeps the wait off SP where it would block xrtT loads
                nc.scalar.dma_start(out=yrt[ds(C * e + r * P, P), :], in_=y)

            # combine: per-expert accumulation paced by the host-computed
            # slot bounds, overlapped with the same expert's compute
            acc = {}
            n_comb = [0]

            def s_gather1(e, tt):
                ge = gp.tile([P, D], BF16, tag="g", name=f"g_{e}_{tt}")
                if n_comb[0] < 4:  # must cover every gp pool rotation
                    # first pool rotations read uninitialized SBUF: OOB-skipped
                    # gather rows would otherwise hold junk (NaN risk) that
                    # w=0 cannot neutralize
                    nc.vector.memset(ge, 0.0)
                n_comb[0] += 1
                nc.gpsimd.indirect_dma_start(
                    out=ge,
                    out_offset=None,
                    in_=yrt[:, :],
                    in_offset=bass.IndirectOffsetOnAxis(
                        ap=giEv[:, e, tt : tt + 1], axis=0
                    ),
                    bounds_check=E * C - 1,
                    oob_is_err=False,
                )
                return ge

            def s_comb1(e, tt, ge):
                t = combp.tile([P, D], BF16, tag="t", name=f"t_{e}_{tt}")
                nc.scalar.activation(
                    out=t, in_=ge, func=AF.Identity,
                    scale=w_sb[:, tt, e : e + 1],
                )
                if e == 0:
                    a = accp.tile([P, D], BF16, tag="acc", name=f"acc_{tt}")
                    nc.vector.tensor_tensor(
                        out=a, in0=t, in1=xbf[:, tt, :], op=ALU.add
                    )
                    acc[tt] = a
                elif e < E - 1:
                    nc.vector.tensor_tensor(
                        out=acc[tt], in0=acc[tt], in1=t, op=ALU.add
                    )
                else:
                    o = combp.tile([P, D], F32, tag="o", name=f"o_{tt}")
                    nc.vector.tensor_tensor(out=o, in0=acc[tt], in1=t, op=ALU.add)
                    nc.sync.dma_start(out=out_d[ts(tt, P), :], in_=o)

            # combine schedule: (e, tt) runs at loop index e*R + pace
            comb_at = {}
            for e in range(E):
                for k in range(TT // 2):
                    lag = 2 if e == 0 else 0  # keep Pool quiet in the ramp
                    jj = e * R + min(max(int(pace[e][k]) + lag, 1), R + lag)
                    comb_at.setdefault(jj, []).append((e, 2 * k))
                    comb_at.setdefault(jj, []).append((e, 2 * k + 1))

            s_load(0)
            s_load(1)
            for j in range(NIT + 1):
                if j + 2 < NIT:
                    s_load(j + 2)
                # pipeline-fill iterations: the ready Z(j) must not sit
                # behind T_u(j-1), which still waits on the first LN chain
                if j in (1, 2):
                    s_z(j)
                    s_tu(j - 1)
                else:
                    if j >= 1:
                        s_tu(j - 1)
                    if j < NIT:
                        s_z(j)
                if j >= 1:
                    s_z2(j - 1)
                if j < NIT:
                    s_ln1(j)
                if j >= 1:
                    s_ln2(j - 1)
                # gathers before combines: an out-write emitted earlier would
                # falsely serialize later (DRAM-aliasing) indirect gathers
                pend = comb_at.get(j, [])
                ges = [s_gather1(e, tt) for e, tt in pend]
                for (e, tt), ge in zip(pend, ges):
                    s_comb1(e, tt, ge)
                if j < NIT:
                    e, r = divmod(j, R)
                    if r == min(5, R - 1) and 2 <= e + 1 < E:
                        load_w(e + 1, 1)
                        load_bve(e + 1)
                        load_reps(e + 1)
                    if r == min(6, R - 1) and 2 <= e + 1 < E:
                        load_w(e + 1, 2)

            # leftover combines (pace == R for the last expert's last tiles)
            for jj in sorted(k for k in comb_at if k > NIT):
                pend = comb_at[jj]
                ges = [s_gather1(e, tt) for e, tt in pend]
                for (e, tt), ge in zip(pend, ges):
                    s_comb1(e, tt, ge)

            utp_ctx.__exit__(None, None, None)
            z2p_ctx.__exit__(None, None, None)
            zp_ctx.__exit__(None, None, None)

    nc.compile()
    return nc


_nc_cache = {}
_nc_lock = threading.Lock()


def _get_nc(T, C, pace, scdep, rcap, flags, num_devices):
    key = (T, C, pace, scdep, rcap, flags, num_devices)
    with _nc_lock:
        if key not in _nc_cache:
            _nc_cache[key] = build_moe_nc(
                T, C, pace, scdep, rcap, flags, num_devices
            )
        return _nc_cache[key]


def _route_stats(x, gate_W, gate_b):
    """Capacity C and combine pacing (host-side shape/schedule decisions
    only -- all routing happens on-device). Tile-major slot order must
    match the device computation."""
    B, N, _ = x.shape
    TT = N // P
    mx = 0
    pace = np.zeros((E, TT), int)
    mincum = None
    for b in range(B):
        gs = x[b].astype(np.float32) @ gate_W + gate_b
        top2 = np.argpartition(-gs, 2, axis=-1)[:, :2]
        sel = np.zeros((N, E), bool)
        for k in range(2):
            sel[np.arange(N), top2[:, k]] = True
        selt = sel.reshape(TT, P, E)
        csum = selt.sum(axis=1).cumsum(axis=0)
        mincum = csum if mincum is None else np.minimum(mincum, csum)
        mx = max(mx, int(csum[-1].max()))
        need = (csum + P - 1) // P  # routed tiles needed after token tile tt
        pace = np.maximum(pace, need.T)
    C = ((mx + 64 + P - 1) // P) * P
    pace = np.minimum(pace, C // P)
    pace_pairs = tuple(
        tuple(int(max(pace[e, 2 * k], pace[e, 2 * k + 1])) for k in range(TT // 2))
        for e in range(E)
    )
    # scatter->routed-tile completion map from the min-over-cores coverage
    scdep = -np.ones((E, TT), int)
    rcap = np.zeros(E, int)
    rlast = np.zeros(E, int)
    for e in range(E):
        prev = 0
        for tt in range(TT):
            v = int(mincum[tt, e]) // P
            if v > prev:
                scdep[e, tt] = v - 1
                rcap[e] = v - 1
            prev = v
        # the expert's LAST scatter must be waitable by loads of routed
        # tiles beyond the min-core coverage (bigger cores still write
        # there); map it explicitly if the min-core coverage didn't
        if scdep[e, TT - 1] < 0:
            scdep[e, TT - 1] = rcap[e] + 1
        rlast[e] = scdep[e, TT - 1]
    return (
        C,
        pace_pairs,
        tuple(tuple(int(x) for x in row) for row in scdep),
        tuple((int(a), int(b)) for a, b in zip(rcap, rlast)),
    )


def kernel(**inputs) -> np.ndarray:
    from concourse.bass_utils import run_bass_kernel_spmd

    x = np.ascontiguousarray(np.asarray(inputs["x"], dtype=np.float32))
    B, N, Dd = x.shape
    assert Dd == D and B == N_CORES, (B, N, Dd)
    weights = {
        k: np.ascontiguousarray(np.asarray(inputs[k], dtype=np.float32))
        for k in (
            "gate_W",
            "gate_b",
            "W1",
            "b1",
            "g1",
            "be1",
            "W2",
            "b2",
            "g2",
            "be2",
        )
    }
    flags = []
    if np.any(weights["gate_b"] != 0):
        flags.append("gb")
    for nm in ("b1", "b2", "be1", "be2"):
        if np.any(weights[nm] != 0):
            flags.append(nm)
    for nm in ("g1", "g2"):
        if np.any(weights[nm] != 1):
            flags.append(nm)
    C, pace, scdep, rcap = _route_stats(x, weights["gate_W"], weights["gate_b"])
    nc = _get_nc(N, C, pace, scdep, rcap, tuple(sorted(flags)), N_CORES)
    in_maps = [dict(weights, x=x[i]) for i in range(N_CORES)]
    res = run_bass_kernel_spmd(nc, in_maps, core_ids=list(range(N_CORES)))
    out = np.stack([r["out"] for r in res.results], axis=0)
    return out.astype(np.float32)
